# revision 30
# baseline (speedup 1.0000x reference)
"""Trainium2 Bass kernel for a BERT decoder layer (no-memory-untied variant).

Distribution: 8 NeuronCores. Core c handles batch element b=c//2 and
sequence-column half r=c%2 (interleaved 64-column blocks of both the decoder
and encoder sequences).  K/V projections are computed over the full sequence
on both cores of a pair (duplicated; collectives are far more expensive than
the duplicated flops here); everything else is column-local, so the kernel
has no communication at all.

Precision split (validated against the fp32 reference):
- SELF attention path entirely fp16: causal early queries average over few
  keys, so fp8 noise in q/k/v would pass straight through to the output.
- CROSS attention path entirely fp8e4 with DoubleRow matmuls (0.5
  cycles/row): probs are near-uniform over 1024 keys, so fp8 noise washes
  out.  K/V/Q-cross projections and the ctx matmuls all ride DoubleRow.
- Output dense fp16 with the LN2 affine folded in afterwards:
  z3 = h_raw*isd2 + (u2*isd2)*(-wosum-1), where wo carries +I on the host
  so h_raw already includes the +z2 residual, and wosum is the
  host-precomputed column sum of the output weights.

Scheduling: scores are computed transposed with a ones-column appended to V
so softmax normalization folds into the ctx matmul; projections interleave
into the attention windows (self window is DVE/ACT-bound, cross window is
ACT-bound on the exp); LN sums ride spare PSUM slots; output DMA is chunked.
"""
import sys

sys.path.insert(0, '/opt/trn_rl_repo')

import contextlib

import numpy as np
import ml_dtypes

import concourse.bass as bass
from concourse import bacc
import concourse.tile as tile
from concourse import mybir
from concourse.bass_utils import run_bass_kernel_spmd

F8 = mybir.dt.float8e4
F16 = mybir.dt.float16
F32 = mybir.dt.float32
EXP = mybir.ActivationFunctionType.Exp
SQRT = mybir.ActivationFunctionType.Sqrt
SQUARE = mybir.ActivationFunctionType.Square
COPY = mybir.ActivationFunctionType.Copy
DR = mybir.MatmulPerfMode.DoubleRow

N, LT, D, H, HD = 4, 1024, 1024, 16, 64
EPS = 1e-12
P = 128
NT = D // P          # 8 d-tiles
W = 512              # per-core column count
KT = LT // P         # 8 k-tiles (full sequence)
NP = NT // 2         # 4 dit-pairs
SCALE = float(1.0 / np.sqrt(HD))

_CACHE = {}
LAST_RESULT = None


# --------------------------------------------------------------------------
# device kernel construction
# --------------------------------------------------------------------------

def _build_nc(with_ln_wb, with_bias):
    from concourse.alu_op_type import AluOpType

    nc = bacc.Bacc("TRN2", target_bir_lowering=False, debug=False,
                   num_devices=8)

    # ---- I/O ----
    x8own_d = nc.declare_dram_parameter("x8own", [P, NT, W], F8,
                                        isOutput=False)
    x8_d = nc.declare_dram_parameter("x8", [P, NT, LT], F8, isOutput=False)
    x16k_d = nc.declare_dram_parameter("x16k", [P, NT, 256], F16,
                                       isOutput=False)
    xh_d = nc.declare_dram_parameter("xh", [P, NT, W], F16, isOutput=False)
    enc8_d = nc.declare_dram_parameter("enc8", [P, NT, LT], F8,
                                       isOutput=False)
    mk_d = nc.declare_dram_parameter("masks", [P, 64], F16, isOutput=False)
    mk8_d = nc.declare_dram_parameter("masks8", [P, 64], F8, isOutput=False)
    wts = {}
    for nm in ["wq16", "wk16"]:
        wts[nm] = nc.declare_dram_parameter(nm, [NT, P, NT, P], F16,
                                            isOutput=False)
    for nm in ["wq8", "wk8", "wqc8", "wkc8"]:
        wts[nm] = nc.declare_dram_parameter(nm, [NT, P, NT, P], F8,
                                            isOutput=False)
    wts["wv16"] = nc.declare_dram_parameter("wv16", [2, P, NT, W], F16,
                                            isOutput=False)
    for nm in ["wv8", "wvc8"]:
        wts[nm] = nc.declare_dram_parameter(nm, [2, P, NT, W], F8,
                                            isOutput=False)
    wo_d = nc.declare_dram_parameter("wo", [NT, P, NT, P], F16,
                                     isOutput=False)
    wos_d = nc.declare_dram_parameter("wos", [P, NT], F32, isOutput=False)
    if with_ln_wb:
        lnp_d = nc.declare_dram_parameter("lnp", [6, P, NT], F32,
                                          isOutput=False)
    if with_bias:
        bia_d = nc.declare_dram_parameter("bias", [7, P, NT], F32,
                                          isOutput=False)
        vbf_d = nc.declare_dram_parameter("vbflat", [2, 1, D], F32,
                                          isOutput=False)
    y_d = nc.declare_dram_parameter("y", [P, NT, W], F16, isOutput=True)

    with tile.TileContext(nc) as tc:
        ctx = contextlib.ExitStack()
        with ctx:
            ctx.enter_context(nc.allow_low_precision(
                reason="fp16 softmax/LN chain validated against fp32 "
                       "reference (rel err < 2e-2 gate)"))
            pool = ctx.enter_context(tc.tile_pool(name="main", bufs=1))
            res16 = ctx.enter_context(tc.tile_pool(name="res16", bufs=2))
            wpool = ctx.enter_context(tc.tile_pool(name="w", bufs=3))
            wvpool = ctx.enter_context(tc.tile_pool(name="wv", bufs=1))
            wv8pool = ctx.enter_context(tc.tile_pool(name="wv8", bufs=2))
            wopool = ctx.enter_context(tc.tile_pool(name="wo", bufs=3))
            espool = ctx.enter_context(tc.tile_pool(name="es", bufs=3))
            ecpool = ctx.enter_context(tc.tile_pool(name="ec", bufs=3))
            smpool = ctx.enter_context(tc.tile_pool(name="sm", bufs=2))
            bczpool = ctx.enter_context(tc.tile_pool(name="bcz", bufs=2))
            bcpool = ctx.enter_context(tc.tile_pool(name="bc", bufs=2))
            statpool = ctx.enter_context(tc.tile_pool(name="stat", bufs=1))
            sqpool = ctx.enter_context(tc.tile_pool(name="sq", bufs=1))
            ps_p = ctx.enter_context(
                tc.tile_pool(name="ps_p", bufs=2, space="PSUM"))
            ps_s = ctx.enter_context(
                tc.tile_pool(name="ps_s", bufs=2, space="PSUM"))
            ps_c = ctx.enter_context(
                tc.tile_pool(name="ps_c", bufs=1, space="PSUM"))

            # ---- constants / small inputs ----
            consts = pool.tile([P, 4], F16, tag="consts")
            nc.vector.memset(consts[:, 0:1], 1.0)
            ones16 = consts[:, 0:1]
            epsc = pool.tile([1, 1], F32, tag="eps")
            nc.vector.memset(epsc[:], EPS)

            masks = pool.tile([P, 64], F16, tag="masks")
            masks8 = pool.tile([P, 64], F8, tag="masks8")

            if with_ln_wb:
                lnt = pool.tile([6, P, NT], F32, tag="lnp")
                nc.sync.dma_start(lnt[:], lnp_d[:])
            if with_bias:
                bt = pool.tile([7, P, NT], F32, tag="bias")
                nc.sync.dma_start(bt[:], bia_d[:])
                vbt = pool.tile([1, 2, D], F32, tag="vbias")
                nc.sync.dma_start(vbt[:], vbf_d.rearrange("a b c -> b a c"))

            # fp8 self-path inputs first: P3a starts ~3us in
            x8own = pool.tile([P, NT, W], F8, tag="x8own")
            nc.sync.dma_start(x8own[:, 0:4, :], x8own_d[:, 0:4, :])
            x8 = pool.tile([P, NT, LT], F8, tag="x8")
            xh = pool.tile([P, NT, W], F16, tag="xh")
            x16k = pool.tile([P, NT, 256], F16, tag="x16k")
            enc8 = pool.tile([P, NT, LT], F8, tag="enc8")
            wos = pool.tile([P, NT], F32, tag="wos")

            # ---------- helpers ----------
            def evac_copy(i, dst, src):
                if i % 2 == 0:
                    nc.vector.tensor_copy(dst, src)
                else:
                    nc.scalar.copy(dst, src)

            def ln_chain(s1, s2, do_iub, act_assist=False):
                """LN stats chain; act_assist moves u/u^2 to ACT (only
                worthwhile when ACT is idle, i.e. the LN3 tail)."""
                u = statpool.tile([1, W], F32, tag="u")
                m2 = statpool.tile([1, W], F32, tag="m2")
                uu = statpool.tile([1, W], F32, tag="uu")
                if act_assist:
                    nc.scalar.activation(u[:], s1, COPY, scale=1.0 / D)
                else:
                    nc.vector.tensor_scalar_mul(u[:], s1, 1.0 / D)
                nc.vector.tensor_scalar(m2[:], s2, 1.0 / D, EPS,
                                        op0=AluOpType.mult,
                                        op1=AluOpType.add)
                if act_assist:
                    nc.scalar.activation(uu[:], u[:], SQUARE)
                else:
                    nc.vector.tensor_mul(uu[:], u[:], u[:])
                nc.vector.tensor_sub(m2[:], m2[:], uu[:])  # var, in place
                nc.vector.reciprocal(m2[:], m2[:])         # 1/var
                isd = statpool.tile([1, W], F32, tag="uu")
                nc.scalar.activation(isd[:], m2[:], SQRT)
                nslots = 3 if do_iub else 2
                pk = statpool.tile([1, 3, W], F16, tag="pk")
                nc.vector.tensor_copy(pk[0:1, 0, :], u[:])
                nc.scalar.copy(pk[0:1, 1, :], isd[:])
                if do_iub:
                    iu = statpool.tile([1, W], F32, tag="m2")
                    nc.vector.tensor_mul(iu[:], u[:], isd[:])
                    nc.vector.tensor_copy(pk[0:1, 2, :], iu[:])
                bc = bcpool.tile([P, 3, W], F16, tag="bc")
                nc.gpsimd.partition_broadcast(bc[:, 0:nslots, :],
                                              pk[:, 0:nslots, :])
                ub, sb = bc[:, 0, :], bc[:, 1, :]
                iub = bc[:, 2, :] if do_iub else None
                return ub, sb, iub

            def ln_apply(z, out, ub, sb, ln_idx, dt):
                """out[:,dt,:] = (z[:,dt,:]-ub)*sb (+ln w/b)."""
                nc.vector.tensor_sub(out[:, dt, :], z[:, dt, :], ub[:])
                nc.vector.tensor_mul(out[:, dt, :], out[:, dt, :], sb[:])
                if with_ln_wb:
                    nc.vector.tensor_scalar(
                        out[:, dt, :], out[:, dt, :],
                        lnt[2 * ln_idx, :, dt:dt + 1],
                        lnt[2 * ln_idx + 1, :, dt:dt + 1],
                        op0=AluOpType.mult, op1=AluOpType.add)

            def softmax_tail(hp, cps, out_z):
                """1/Z applied straight from PSUM -> out_z[:,hp,:]."""
                invz = smpool.tile([1, 2, KT, 64], F16, tag="invz")
                nc.vector.reciprocal(invz[0:1], cps[64:65])
                izb = bczpool.tile([64, 2, KT, 64], F16, tag="izb")
                nc.gpsimd.partition_broadcast(izb[:], invz[:])
                cpf = cps[:].rearrange("p a b c -> p a (b c)")
                izf = izb[:].rearrange("p a b c -> p a (b c)")
                nc.vector.tensor_mul(out_z[0:64, hp, :], cpf[0:64, 0, :],
                                     izf[:, 0, :])
                nc.vector.tensor_mul(out_z[64:P, hp, :], cpf[0:64, 1, :],
                                     izf[:, 1, :])

            # =========== P1: Q8 self projection (fp8 DR) ===================
            def _late_dmas():
                nc.sync.dma_start(masks8[:], mk8_d[:])
                nc.sync.dma_start(x8[:, 0:4, :], x8_d[:, 0:4, :])
                nc.sync.dma_start(x8[:, 4:8, :], x8_d[:, 4:8, :])

            qt = res16.tile([P, NT, W], F16, tag="q16")
            for dot in range(NT):
                wt = wpool.tile([P, NT, P], F8, tag="w8")
                nc.sync.dma_start(wt[:], wts["wq8"][dot])
                if dot == 0:
                    nc.sync.dma_start(x8own[:, 4:8, :], x8own_d[:, 4:8, :])
                if dot == 2:
                    _late_dmas()
                ps = ps_p.tile([P, W], F32, tag="pp")
                for j in range(NP):
                    nc.tensor.matmul(ps[:], wt[:, 2 * j:2 * j + 2, :],
                                     x8own[:, 2 * j:2 * j + 2, :],
                                     start=(j == 0), stop=(j == NP - 1),
                                     perf_mode=DR)
                if with_bias:
                    nc.vector.tensor_scalar_add(qt[:, dot, :], ps[:],
                                                bt[0, :, dot:dot + 1])
                else:
                    evac_copy(dot, qt[:, dot, :], ps[:])

            # =========== K8/V8 self (fp8 DR), interleaved with attention ===
            ktf = pool.tile([P, NT, LT], F16, tag="ktf")

            def kdot8(dot):
                wt = wpool.tile([P, NT, P], F8, tag="w8")
                nc.sync.dma_start(wt[:], wts["wk8"][dot])
                for blk in range(2):
                    ps = ps_p.tile([P, W], F32, tag="pp")
                    for j in range(NP):
                        nc.tensor.matmul(
                            ps[:], wt[:, 2 * j:2 * j + 2, :],
                            x8[:, 2 * j:2 * j + 2, blk * W:(blk + 1) * W],
                            start=(j == 0), stop=(j == NP - 1), perf_mode=DR)
                    dstv = ktf[:, dot, blk * W:(blk + 1) * W]
                    if with_bias:
                        nc.vector.tensor_scalar_add(dstv, ps[:],
                                                    bt[1, :, dot:dot + 1])
                    else:
                        evac_copy(dot + blk, dstv, ps[:])

            vtf = pool.tile([P, KT, 16, 65], F8, tag="vtf")
            nc.vector.memset(vtf[:, :, :, 64:65], 1.0)

            def vblk8(blk):
                wt = wv8pool.tile([P, NT, W], F8, tag="wv8")
                nc.sync.dma_start(wt[:], wts["wv8"][blk])
                for lt in range(KT):
                    ps = ps_p.tile([P, W], F32, tag="pp")
                    for j in range(NP):
                        nc.tensor.matmul(
                            ps[:], x8[:, 2 * j:2 * j + 2, bass.ts(lt, P)],
                            wt[:, 2 * j:2 * j + 2, :],
                            start=(j == 0), stop=(j == NP - 1),
                            perf_mode=DR)
                    dstv = vtf[:, lt, 8 * blk:8 * (blk + 1), 0:64]
                    psv = ps[:].rearrange("p (h c) -> p h c", c=64)
                    if with_bias:
                        bb = bcpool.tile([P, W], F32, tag="vbb")
                        nc.gpsimd.partition_broadcast(
                            bb[:], vbt[0:1, 0, blk * W:(blk + 1) * W])
                        nc.vector.tensor_add(
                            dstv, psv,
                            bb[:].rearrange("p (h c) -> p h c", c=64))
                    else:
                        evac_copy(blk + lt, dstv, psv)

            kdot8(0)
            vblk8(0)

            # fp16 early-query projections (weights arrive mid-window)
            qt16 = pool.tile([P, NT, 128], F16, tag="qt16")
            ktf16 = pool.tile([P, NT, 256], F16, tag="ktf16")
            vtf16 = pool.tile([P, 2, 16, 65], F16, tag="vtf16")
            nc.vector.memset(vtf16[:, :, :, 64:65], 1.0)

            def q16proj():
                for dot in range(NT):
                    wt = wpool.tile([P, NT, P], F16, tag="w16")
                    nc.sync.dma_start(wt[:], wts["wq16"][dot])
                    ps = ps_p.tile([P, W], F32, tag="pp")
                    for dit in range(NT):
                        nc.tensor.matmul(ps[0:P, 0:128], wt[:, dit, :],
                                         xh[:, dit, 0:128],
                                         start=(dit == 0),
                                         stop=(dit == NT - 1))
                    if with_bias:
                        nc.vector.tensor_scalar_add(
                            qt16[:, dot, :], ps[0:P, 0:128],
                            bt[0, :, dot:dot + 1])
                    else:
                        evac_copy(dot, qt16[:, dot, :], ps[0:P, 0:128])

            def k16proj():
                for dot in range(NT):
                    wt = wpool.tile([P, NT, P], F16, tag="w16")
                    nc.sync.dma_start(wt[:], wts["wk16"][dot])
                    ps = ps_p.tile([P, W], F32, tag="pp")
                    for dit in range(NT):
                        nc.tensor.matmul(ps[0:P, 0:256], wt[:, dit, :],
                                         x16k[:, dit, :],
                                         start=(dit == 0),
                                         stop=(dit == NT - 1))
                    if with_bias:
                        nc.vector.tensor_scalar_add(
                            ktf16[:, dot, :], ps[0:P, 0:256],
                            bt[1, :, dot:dot + 1])
                    else:
                        evac_copy(dot, ktf16[:, dot, :], ps[0:P, 0:256])

            def v16proj(blk):
                wt = wvpool.tile([P, NT, W], F16, tag="wv16")
                nc.sync.dma_start(wt[:], wts["wv16"][blk])
                for lt in range(2):
                    ps = ps_p.tile([P, W], F32, tag="pp")
                    for dit in range(NT):
                        nc.tensor.matmul(
                            ps[:], x16k[:, dit, bass.ts(lt, P)],
                            wt[:, dit, :],
                            start=(dit == 0), stop=(dit == NT - 1))
                    dstv = vtf16[:, lt, 8 * blk:8 * (blk + 1), 0:64]
                    psv = ps[:].rearrange("p (h c) -> p h c", c=64)
                    if with_bias:
                        bb = bcpool.tile([P, W], F32, tag="vbb")
                        nc.gpsimd.partition_broadcast(
                            bb[:], vbt[0:1, 0, blk * W:(blk + 1) * W])
                        nc.vector.tensor_add(
                            dstv, psv,
                            bb[:].rearrange("p (h c) -> p h c", c=64))
                    else:
                        evac_copy(blk + lt, dstv, psv)

            # =========== P3a: self attention, late queries (fp8 path) ======
            z1 = res16.tile([P, NT, W], F16, tag="res16")
            sqf = sqpool.tile([P, NT, W], F16, tag="sqf")
            maskbc = masks[:, None, :].to_broadcast((P, 2, 64))
            maskbc8 = masks8[:, None, :].to_broadcast((P, 2, 64))

            def softmax_tail_r(hp, cps, out_z, qb0, qb1):
                """1/Z from PSUM for query blocks [qb0, qb1)."""
                nq = qb1 - qb0
                invz = smpool.tile([1, 2, KT, 64], F16, tag="invz")
                nc.vector.reciprocal(invz[0:1, :, qb0:qb1, :],
                                     cps[64:65, :, qb0:qb1, :])
                izb = bczpool.tile([64, 2, KT, 64], F16, tag="izb")
                nc.gpsimd.partition_broadcast(izb[:, :, qb0:qb1, :],
                                              invz[:, :, qb0:qb1, :])
                cs = slice(64 * qb0, 64 * qb1)
                cpf = cps[:, :, qb0:qb1, :].rearrange("p a b c -> p a (b c)")
                izf = izb[:, :, qb0:qb1, :].rearrange("p a b c -> p a (b c)")
                nc.vector.tensor_mul(out_z[0:64, hp, cs], cpf[0:64, 0, :],
                                     izf[:, 0, :])
                nc.vector.tensor_mul(out_z[64:P, hp, cs], cpf[0:64, 1, :],
                                     izf[:, 1, :])

            def _ctx8(hp, qb, e, cps):
                npair = (qb + 2) // 2
                for h in range(2):
                    for j in range(npair):
                        nc.tensor.matmul(
                            cps[:, h, qb, :],
                            vtf[:, 2 * j:2 * j + 2, 2 * hp + h, 0:65],
                            e[:, h, 2 * j:2 * j + 2, :],
                            start=(j == 0), stop=(j == npair - 1),
                            perf_mode=DR)

            def self_hp_a(hp):
                cps = ps_c.tile([65, 2, KT, 64], F32, tag="cps")
                pend = None
                for qb in range(2, KT):
                    nk = qb + 1
                    qs = slice(64 * qb, 64 * qb + 64)
                    sc = ps_s.tile([P, 2, KT, 64], F32, tag="sc")
                    for t in range(nk):
                        nc.tensor.matmul(
                            sc[:, 0, t, :], ktf[0:64, hp, bass.ts(t, P)],
                            qt[0:64, hp, qs], start=True, stop=True)
                        nc.tensor.matmul(
                            sc[:, 1, t, :], ktf[64:P, hp, bass.ts(t, P)],
                            qt[64:P, hp, qs], start=True, stop=True)
                    e = espool.tile([P, 2, KT, 64], F8, tag="es8")
                    nc.scalar.activation(e[:, :, 0:nk, :], sc[:, :, 0:nk, :],
                                         EXP, scale=SCALE)
                    nc.vector.tensor_mul(e[:, :, qb, :], e[:, :, qb, :],
                                         maskbc8)
                    if nk % 2 == 1:
                        nc.vector.memset(e[:, :, nk, :], 0.0)
                    if pend is not None:
                        _ctx8(hp, *pend, cps)
                    pend = (qb, e)
                _ctx8(hp, *pend, cps)
                softmax_tail_r(hp, cps, z1, 2, KT)

            kc_pre = {}
            vc_pre = {}
            for hp in range(NT):
                if hp + 1 < NT:
                    kdot8(hp + 1)
                if hp == 1:
                    vblk8(1)
                if hp == 2:
                    nc.sync.dma_start(xh[:, 0:4, :], xh_d[:, 0:4, :])
                    nc.sync.dma_start(xh[:, 4:8, :], xh_d[:, 4:8, :])
                    nc.sync.dma_start(x16k[:], x16k_d[:])
                    nc.sync.dma_start(masks[:], mk_d[:])
                if hp == 3:
                    q16proj()
                if hp == 4:
                    k16proj()
                if hp == 5:
                    v16proj(0)
                    v16proj(1)
                self_hp_a(hp)
                if hp >= 5:
                    wt = wpool.tile([P, NT, P], F8, tag="w8")
                    nc.sync.dma_start(wt[:], wts["wkc8"][hp - 5])
                    kc_pre[hp - 5] = wt
                if hp == 6:
                    nc.sync.dma_start(enc8[:, 0:4, :], enc8_d[:, 0:4, :])
                    nc.sync.dma_start(enc8[:, 4:8, :], enc8_d[:, 4:8, :])
                    nc.sync.dma_start(wos[:], wos_d[:])
                    wt = wv8pool.tile([P, NT, W], F8, tag="wv8")
                    nc.sync.dma_start(wt[:], wts["wvc8"][0])
                    vc_pre[0] = wt

            def kcdot(dot):
                if dot in kc_pre:
                    wt = kc_pre[dot]
                else:
                    wt = wpool.tile([P, NT, P], F8, tag="w8")
                    nc.sync.dma_start(wt[:], wts["wkc8"][dot])
                for blk in range(2):
                    ps = ps_p.tile([P, W], F32, tag="pp")
                    for j in range(NP):
                        nc.tensor.matmul(
                            ps[:], wt[:, 2 * j:2 * j + 2, :],
                            enc8[:, 2 * j:2 * j + 2, blk * W:(blk + 1) * W],
                            start=(j == 0), stop=(j == NP - 1), perf_mode=DR)
                    dstv = ktfe[:, dot, blk * W:(blk + 1) * W]
                    if with_bias:
                        nc.vector.tensor_scalar_add(dstv, ps[:],
                                                    bt[5, :, dot:dot + 1])
                    else:
                        evac_copy(dot + blk, dstv, ps[:])

            def vcblk8(blk, pre_wt=None):
                if pre_wt is not None:
                    wt = pre_wt
                else:
                    wt = wv8pool.tile([P, NT, W], F8, tag="wv8")
                    nc.sync.dma_start(wt[:], wts["wvc8"][blk])
                for lt in range(KT):
                    ps = ps_p.tile([P, W], F32, tag="pp")
                    for j in range(NP):
                        nc.tensor.matmul(
                            ps[:], enc8[:, 2 * j:2 * j + 2, bass.ts(lt, P)],
                            wt[:, 2 * j:2 * j + 2, :],
                            start=(j == 0), stop=(j == NP - 1),
                            perf_mode=DR)
                    dstv = vtfe[:, lt, 8 * blk:8 * (blk + 1), 0:64]
                    psv = ps[:].rearrange("p (h c) -> p h c", c=64)
                    if with_bias:
                        bb = bcpool.tile([P, W], F32, tag="vbb")
                        nc.gpsimd.partition_broadcast(
                            bb[:], vbt[0:1, 1, blk * W:(blk + 1) * W])
                        nc.vector.tensor_add(
                            dstv, psv,
                            bb[:].rearrange("p (h c) -> p h c", c=64))
                    else:
                        evac_copy(blk + lt, dstv, psv)


            # =========== P3b: early queries (fp16 fixup pass) ==============
            # K-cross dots and V-cross blk0 interleave here (they only need
            # enc8 + their weights); z1 residual/squares follow each hp
            ktfe = pool.tile([P, NT, LT], F16, tag="ktfe")
            vtfe = pool.tile([P, KT, 16, 65], F8, tag="vtfe")
            nc.vector.memset(vtfe[:, :, :, 64:65], 1.0)
            def p3b_front(hp):
                if hp % 2 == 0:
                    cps = ps_c.tile([65, 2, KT, 64], F32, tag="cps")
                    cpsv = cps[:, :, 0:2, :]
                else:
                    pp = ps_p.tile([P, W], F32, tag="pp")
                    cpsv = pp[:].rearrange("p (a b c) -> p a b c",
                                           a=2, b=4, c=64)[:, :, 0:2, :]
                for qb in range(2):
                    nk = qb + 1
                    qs = slice(64 * qb, 64 * qb + 64)
                    sc = ps_s.tile([P, 2, KT, 64], F32, tag="sc")
                    for t in range(nk):
                        nc.tensor.matmul(
                            sc[:, 0, t, :], ktf16[0:64, hp, bass.ts(t, P)],
                            qt16[0:64, hp, qs], start=True, stop=True)
                        nc.tensor.matmul(
                            sc[:, 1, t, :], ktf16[64:P, hp, bass.ts(t, P)],
                            qt16[64:P, hp, qs], start=True, stop=True)
                    e = espool.tile([P, 2, 2, 64], F16, tag="es16")
                    nc.scalar.activation(e[:, :, 0:nk, :], sc[:, :, 0:nk, :],
                                         EXP, scale=SCALE)
                    nc.vector.tensor_mul(e[:, :, qb, :], e[:, :, qb, :],
                                         maskbc)
                    for h in range(2):
                        for t in range(nk):
                            nc.tensor.matmul(
                                cpsv[0:65, h, qb, :],
                                vtf16[:, t, 2 * hp + h, 0:65],
                                e[:, h, t, :],
                                start=(t == 0), stop=(t == qb))
                return cpsv

            def p3b_tail(hp, cpsv):
                nq = 2
                invz = smpool.tile([1, 2, KT, 64], F16, tag="invz")
                nc.vector.reciprocal(invz[0:1, :, 0:nq, :],
                                     cpsv[64:65, :, 0:nq, :])
                izb = bczpool.tile([64, 2, KT, 64], F16, tag="izb")
                nc.gpsimd.partition_broadcast(izb[:, :, 0:nq, :],
                                              invz[:, :, 0:nq, :])
                cs = slice(0, 64 * nq)
                cpf = cpsv[:, :, 0:nq, :].rearrange("p a b c -> p a (b c)")
                izf = izb[:, :, 0:nq, :].rearrange("p a b c -> p a (b c)")
                nc.vector.tensor_mul(z1[0:64, hp, cs], cpf[0:64, 0, :],
                                     izf[:, 0, :])
                nc.vector.tensor_mul(z1[64:P, hp, cs], cpf[0:64, 1, :],
                                     izf[:, 1, :])
                nc.vector.tensor_add(z1[:, hp, :], z1[:, hp, :],
                                     xh[:, hp, :])
                nc.gpsimd.tensor_mul(sqf[:, hp, :], z1[:, hp, :],
                                     z1[:, hp, :])

            pend3 = None
            for hp in range(NT):
                cpsv = p3b_front(hp)
                if pend3 is not None:
                    p3b_tail(*pend3)
                pend3 = (hp, cpsv)
                if hp < 6:
                    kcdot(hp)
                if hp == 6:
                    vcblk8(0, pre_wt=vc_pre.get(0))
            p3b_tail(*pend3)

            # =========== P4: LN1 + Q-cross ================================

            kcdot(6)
            kcdot(7)

            # LN1 sums in a ps_s slot
            sA = ps_s.tile([P, 2, KT, 64], F32, tag="sc")
            s1a = sA[0:1, 0].rearrange("p a b -> p (a b)")
            s2a = sA[0:1, 1].rearrange("p a b -> p (a b)")
            for dt in range(NT):
                nc.tensor.matmul(s1a, ones16, z1[:, dt, :],
                                 start=(dt == 0), stop=(dt == NT - 1))
            for dt in range(NT):
                nc.tensor.matmul(s2a, ones16, sqf[:, dt, :],
                                 start=(dt == 0), stop=(dt == NT - 1))

            ub1, sb1, _ = ln_chain(s1a, s2a, False)
            a16 = pool.tile([P, NT, W], F16, tag="a16")
            a8 = pool.tile([P, NT, W], F8, tag="a8")
            for dt in range(NT):
                ln_apply(z1, a16, ub1, sb1, 0, dt)
                if dt % 2 == 0:
                    nc.scalar.activation(a8[:, dt, :], a16[:, dt, :], COPY)
                else:
                    nc.vector.tensor_copy(a8[:, dt, :], a16[:, dt, :])

            qtc = res16.tile([P, NT, W], F16, tag="q16")
            for dot in range(NT):
                wt = wpool.tile([P, NT, P], F8, tag="w8")
                nc.sync.dma_start(wt[:], wts["wqc8"][dot])
                ps = ps_p.tile([P, W], F32, tag="pp")
                for j in range(NP):
                    nc.tensor.matmul(ps[:], wt[:, 2 * j:2 * j + 2, :],
                                     a8[:, 2 * j:2 * j + 2, :],
                                     start=(j == 0), stop=(j == NP - 1),
                                     perf_mode=DR)
                if with_bias:
                    nc.vector.tensor_scalar_add(qtc[:, dot, :], ps[:],
                                                bt[4, :, dot:dot + 1])
                else:
                    evac_copy(dot, qtc[:, dot, :], ps[:])

            # =========== P5: cross attention (fp8 DR ctx) ==================
            z2 = res16.tile([P, NT, W], F16, tag="res16")
            wo_pre = {}

            for hp in range(NT):
                if hp == 0:
                    vcblk8(1)
                cps = ps_c.tile([65, 2, KT, 64], F32, tag="cps")
                cpsv = cps[:].rearrange("p a b c -> p a (b c)")
                epair = None
                for t in range(KT):
                    sc = ps_s.tile([P, 2, KT, 64], F32, tag="sc")
                    scv = sc[:].rearrange("p a b c -> p a (b c)")
                    nc.tensor.matmul(scv[:, 0, :],
                                     ktfe[0:64, hp, bass.ts(t, P)],
                                     qtc[0:64, hp, :], start=True, stop=True)
                    nc.tensor.matmul(scv[:, 1, :],
                                     ktfe[64:P, hp, bass.ts(t, P)],
                                     qtc[64:P, hp, :], start=True, stop=True)
                    if t % 2 == 0:
                        epair = ecpool.tile([P, 2, 2, W], F8, tag="ec")
                    nc.scalar.activation(epair[:, t % 2, :, :], scv[:],
                                         EXP, scale=SCALE)
                    if t % 2 == 1:
                        j = t // 2
                        for h in range(2):
                            nc.tensor.matmul(
                                cpsv[:, h, :],
                                vtfe[:, 2 * j:2 * j + 2, 2 * hp + h, 0:65],
                                epair[:, :, h, :],
                                start=(j == 0), stop=(j == KT // 2 - 1),
                                perf_mode=DR)
                softmax_tail_r(hp, cps, z2, 0, KT)
                nc.vector.tensor_add(z2[:, hp, :], z2[:, hp, :],
                                     a16[:, hp, :])
                nc.vector.tensor_mul(sqf[:, hp, :], z2[:, hp, :],
                                     z2[:, hp, :])
                if hp >= 5:
                    wt = wopool.tile([P, NT, P], F16, tag="wo16")
                    nc.sync.dma_start(wt[:], wo_d[hp - 5])
                    wo_pre[hp - 5] = wt

            # =========== P6: LN2 sums + chain, then output dense ===========
            sB = ps_s.tile([P, 2, KT, 64], F32, tag="sc")
            s1b = sB[0:1, 0].rearrange("p a b -> p (a b)")
            s2b = sB[0:1, 1].rearrange("p a b -> p (a b)")
            for dt in range(NT):
                nc.tensor.matmul(s1b, ones16, z2[:, dt, :],
                                 start=(dt == 0), stop=(dt == NT - 1))
            for dt in range(NT):
                nc.tensor.matmul(s2b, ones16, sqf[:, dt, :],
                                 start=(dt == 0), stop=(dt == NT - 1))

            ub2, sb2, iub2 = ln_chain(s1b, s2b, True)

            # output dense (wo carries +I so h_raw includes the +z2
            # residual); fold: z3 = h_raw*sb2 + iub2*(-wosum-1)
            z3 = res16.tile([P, NT, W], F16, tag="res16")
            sC = ps_s.tile([P, 2, KT, 64], F32, tag="sc")
            s1c = sC[0:1, 0].rearrange("p a b -> p (a b)")
            s2c = sC[0:1, 1].rearrange("p a b -> p (a b)")
            for dot in range(NT):
                if dot in wo_pre:
                    wt = wo_pre[dot]
                else:
                    wt = wopool.tile([P, NT, P], F16, tag="wo16")
                    nc.sync.dma_start(wt[:], wo_d[dot])
                ps = ps_p.tile([P, W], F32, tag="pp")
                for dit in range(NT):
                    nc.tensor.matmul(ps[:], wt[:, dit, :], z2[:, dit, :],
                                     start=(dit == 0), stop=(dit == NT - 1))
                nc.scalar.copy(z3[:, dot, :], ps[:])
                nc.vector.tensor_mul(z3[:, dot, :], z3[:, dot, :], sb2[:])
                nc.vector.scalar_tensor_tensor(
                    z3[:, dot, :], iub2[:], wos[:, dot:dot + 1],
                    z3[:, dot, :], op0=AluOpType.mult, op1=AluOpType.add)
                if with_bias:
                    nc.vector.tensor_scalar_add(z3[:, dot, :], z3[:, dot, :],
                                                bt[3, :, dot:dot + 1])
                nc.scalar.activation(sqf[:, dot, :], z3[:, dot, :],
                                      SQUARE)
                nc.tensor.matmul(s1c, ones16, z3[:, dot, :],
                                 start=(dot == 0), stop=(dot == NT - 1))
                nc.tensor.matmul(s2c, ones16, sqf[:, dot, :],
                                 start=(dot == 0), stop=(dot == NT - 1))

            # =========== P7: LN3 + output ==================================
            ub3, sb3, _ = ln_chain(s1c, s2c, False, act_assist=True)
            y16 = res16.tile([P, NT, W], F16, tag="res16")
            for g in range(4):
                ln_apply(z3, y16, ub3, sb3, 2, 2 * g)
                ln_apply(z3, y16, ub3, sb3, 2, 2 * g + 1)
                nc.sync.dma_start(y_d[:, 2 * g:2 * g + 2, :],
                                  y16[:, 2 * g:2 * g + 2, :])

    nc.compile()
    return nc


# --------------------------------------------------------------------------
# host-side packing
# --------------------------------------------------------------------------

F8NP = ml_dtypes.float8_e4m3


def _w_pack(w, dtype):
    """torch-Linear weight [dout, din] -> [NT, P, NT, P] (wT blocked)."""
    return _w_packT(np.asarray(w, dtype=np.float32).T, dtype)


def _w_packT(wT, dtype):
    return np.ascontiguousarray(
        wT.reshape(NT, P, NT, P).transpose(2, 1, 0, 3)).astype(dtype)


def _wv_pack(w, dtype):
    """V weight [dout, din] -> [2, P, NT, 512] (wT, dout-major blocks)."""
    wT = np.asarray(w, dtype=np.float32).T
    return np.ascontiguousarray(
        wT.reshape(NT, P, 2, W).transpose(2, 1, 0, 3)).astype(dtype)


def _dout_vec_pack(b):
    """[D] per-dout vector -> [P, NT]."""
    return np.ascontiguousarray(np.asarray(b).reshape(NT, P).T).astype(
        np.float32)


def _feat_full(x, dtype):
    """[L, D] -> [P, NT, L] feature-major."""
    return np.ascontiguousarray(
        np.asarray(x, np.float32).T.reshape(NT, P, LT).transpose(1, 0, 2)
    ).astype(dtype)


def _feat_cols(x, cols, dtype):
    """[L, D] -> [P, NT, W] feature-major for the given columns."""
    xT = np.asarray(x, np.float32).T[:, cols]
    return np.ascontiguousarray(
        xT.reshape(NT, P, W).transpose(1, 0, 2)).astype(dtype)


ln_names = ["n1_w", "n1_b", "n2_w", "n2_b", "out_ln_w", "out_ln_b"]


def _flags(inp):
    dec_mask = inp["dec_mask"]
    enc_mask = inp["enc_mask"]
    if not (np.all(dec_mask == 1.0) and np.all(enc_mask == 1.0)):
        raise NotImplementedError("padding masks not supported")
    with_ln_wb = not all(
        np.all(inp[n] == (1.0 if n.endswith("w") else 0.0)) for n in ln_names)
    if with_ln_wb:
        raise NotImplementedError("LN weight/bias not supported in v2")
    b_names = ["sa_qb", "sa_kb", "sa_vb", "out_b", "ca_qb", "ca_kb", "ca_vb"]
    with_bias = any(np.any(inp[n] != 0.0) for n in b_names)
    return with_ln_wb, with_bias


def _role_cols(r):
    return np.concatenate(
        [np.arange(128 * j + 64 * r, 128 * j + 64 * r + 64) for j in range(8)])


def build_in_maps(inputs):
    inp = {k: np.asarray(v) for k, v in inputs.items()}
    with_ln_wb, with_bias = _flags(inp)

    wmap = {
        "wq16": _w_pack(inp["sa_qw"], np.float16),
        "wk16": _w_pack(inp["sa_kw"], np.float16),
        "wq8": _w_pack(inp["sa_qw"], F8NP),
        "wk8": _w_pack(inp["sa_kw"], F8NP),
        "wqc8": _w_pack(inp["ca_qw"], F8NP),
        "wkc8": _w_pack(inp["ca_kw"], F8NP),
        "wv16": _wv_pack(inp["sa_vw"], np.float16),
        "wv8": _wv_pack(inp["sa_vw"], F8NP),
        "wvc8": _wv_pack(inp["ca_vw"], F8NP),
        "wo": _w_packT(np.asarray(inp["out_w"], np.float32).T
                       + np.eye(D, dtype=np.float32), np.float16),
        "wos": _dout_vec_pack(
            -np.asarray(inp["out_w"], np.float32).sum(axis=1) - 1.0),
    }
    if with_bias:
        wmap["bias"] = np.stack(
            [_dout_vec_pack(inp[n]) for n in
             ["sa_qb", "sa_kb", "sa_vb", "out_b", "ca_qb", "ca_kb", "ca_vb"]])
        wmap["vbflat"] = np.stack(
            [np.asarray(inp["sa_vb"], np.float32).reshape(1, D),
             np.asarray(inp["ca_vb"], np.float32).reshape(1, D)])

    x8_b = [_feat_full(inp["dec_hidden_states"][b], F8NP) for b in range(N)]
    x16k_b = [np.ascontiguousarray(
        _feat_full(inp["dec_hidden_states"][b], np.float16)[:, :, 0:256])
        for b in range(N)]
    e8_b = [_feat_full(inp["enc_outputs"][b], F8NP) for b in range(N)]

    in_maps = []
    for c in range(8):
        b, r = c // 2, c % 2
        cols = _role_cols(r)
        m = (np.arange(P)[:, None] <= 64 * r + np.arange(64)[None, :])
        im = {
            "x8own": _feat_cols(inp["dec_hidden_states"][b], cols, F8NP),
            "x8": x8_b[b],
            "x16k": x16k_b[b],
            "xh": _feat_cols(inp["dec_hidden_states"][b], cols, np.float16),
            "enc8": e8_b[b],
            "masks": np.ascontiguousarray(m).astype(np.float16),
            "masks8": np.ascontiguousarray(m).astype(F8NP),
        }
        im.update(wmap)
        in_maps.append(im)
    return in_maps


def kernel(**inputs):
    inp = {k: np.asarray(v) for k, v in inputs.items()}
    key = _flags(inp)
    if key not in _CACHE:
        _CACHE[key] = _build_nc(*key)
    nc = _CACHE[key]
    in_maps = build_in_maps(inp)

    global LAST_RESULT
    res = run_bass_kernel_spmd(nc, in_maps, list(range(8)))
    LAST_RESULT = res

    out = np.zeros((N, LT, D), dtype=np.float32)
    for c in range(8):
        b, r = c // 2, c % 2
        y = np.asarray(res.results[c]["y"]).astype(np.float32)  # [P, NT, W]
        out[b, _role_cols(r), :] = y.transpose(1, 0, 2).reshape(D, W).T
    return out


if __name__ == "__main__":
    _build_nc(False, False)
    print("built ok")

